# revision 25
# baseline (speedup 1.0000x reference)
"""Trainium2 Bass kernel for segment-mean + linear head + L2-normalize.

Reference computation (per batch element b, frame t):
  mean[s, c]  = mean over pixels p with sp_mask[p] == s of maps[c, p]
  sp[s, d]    = sum_c mean[s, c] * W_fc[d, c]
  out[d, s]   = sp[s, d] / max(||sp[s, :]||_2, 1e-12)

Key identity used: the per-segment count division cancels inside the L2
normalization (normalize(v / n) == normalize(v) for n > 0, and empty
segments are exactly zero either way), so counts are never computed.

Inputs are pre-cast to bf16 on the host (the kernel computes in bf16 either
way, and this halves the HBM read volume -- the kernel is memory-bound).

Pipeline per (b, t) on one NeuronCore (data-parallel over B across 8 cores):
  1. DMA maps[b,:,t] (512 x 4096 bf16, channel-major) -> SBUF, split across
     both HWDGE rings (sync + scalar engines)
  2. stage 1 (PE): proj[p, d] = sum_c feats[c, p] * W[d, c]  (pixel-major out)
  3. one-hot O[p, s] = (sp_mask[p] == s) built on DVE from an iota constant
     (mask reaches chunk-major partition order via a small PE transpose)
  4. stage 2 (PE): seg[s, d] = sum_p O[p, s] * proj[p, d]  (PSUM accumulate)
  5. norm (ACT/DVE): seg / sqrt(sum_d seg^2 + 1e-30), DMA out as (t, s, d);
     the host transposes the gathered result to (d, t, s).

Measured ~58-70 us per clip on trn2 (8 cores in parallel), vs a ~92 us
fp32-input memory roofline and ~52 us bf16-input DMA floor.
"""

import numpy as np

B, C, T, H, W = 8, 512, 4, 64, 64
HW = H * W          # 4096 pixels per frame
N_SP = 100
D_OUT = 128
N_CORES = 8
P = 128             # SBUF partitions
NCH = C // P        # 4 channel chunks
NPIX = HW // P      # 32 pixel chunks

_CACHE = {}


def _build_program(reps=1, feats_bufs=3, dma_split=2, dma_only=False):
    from contextlib import ExitStack

    import concourse.tile as tile
    from concourse import bacc, mybir
    from concourse.masks import make_identity

    f32 = mybir.dt.float32
    bf16 = mybir.dt.bfloat16
    i32 = mybir.dt.int32

    nc = bacc.Bacc(
        "TRN2",
        target_bir_lowering=False,
        debug=False,
        num_devices=N_CORES,
    )

    # maps and W arrive pre-cast to bf16 (host-side) -- the kernel computes in
    # bf16 anyway, so this halves the HBM read volume at identical accuracy
    maps_t = nc.dram_tensor("maps_bf16", [C, T, H, W], bf16, kind="ExternalInput")
    mask_t = nc.dram_tensor("sp_mask", [T, H, W], i32, kind="ExternalInput")
    wfc_t = nc.dram_tensor("W_fcT", [C, D_OUT], bf16, kind="ExternalInput")
    out_t = nc.dram_tensor("out", [T, N_SP, D_OUT], f32, kind="ExternalOutput")

    # DRAM views
    # maps as (c_in_chunk, chunk, t, pixel): partition dim = channel-in-chunk
    maps_r = maps_t.ap().rearrange("(cj c) t h w -> c cj t (h w)", c=P)
    # sp_mask frame as (chunk, pixel_in_chunk): chunk k = pixels [128k, 128k+128)
    mask_r = mask_t.ap().rearrange("t h w -> t (h w)").rearrange(
        "t (k p) -> t k p", p=P
    )
    # W_fc^T per channel chunk: (c_in_chunk, chunk, d)
    wfc_r = wfc_t.ap().rearrange("(cj c) d -> c cj d", c=P)

    with tile.TileContext(nc) as tc, ExitStack() as ctx:
        const_pool = ctx.enter_context(tc.tile_pool(name="const", bufs=1))
        feats_pool = ctx.enter_context(tc.tile_pool(name="feats", bufs=feats_bufs))
        mask_pool = ctx.enter_context(tc.tile_pool(name="mask", bufs=2))
        maskf_pool = ctx.enter_context(tc.tile_pool(name="maskf", bufs=2))
        oall_pool = ctx.enter_context(tc.tile_pool(name="oall", bufs=2))
        proj_pool = ctx.enter_context(tc.tile_pool(name="proj", bufs=4))
        outsb_pool = ctx.enter_context(tc.tile_pool(name="outsb", bufs=2))
        small_pool = ctx.enter_context(tc.tile_pool(name="small", bufs=4))
        pp_pool = ctx.enter_context(tc.tile_pool(name="pp", bufs=3, space="PSUM"))
        seg_pool = ctx.enter_context(tc.tile_pool(name="seg", bufs=2, space="PSUM"))
        mtp_pool = ctx.enter_context(tc.tile_pool(name="mtp", bufs=2, space="PSUM"))

        # iota constant: column k*100+s holds value s (segment id pattern).
        # Values are 0..99, exact in f32.
        iota_tile = const_pool.tile([P, NPIX * N_SP], f32)
        nc.gpsimd.iota(
            iota_tile[:].rearrange("p (k s) -> p k s", s=N_SP),
            pattern=[[0, NPIX], [1, N_SP]],
            base=0,
            channel_multiplier=0,
            allow_small_or_imprecise_dtypes=True,
        )

        identity = const_pool.tile([P, P], f32)
        make_identity(nc, identity[:])

        # tiny positive bias so sqrt(ss + eps) never hits 1/0 on empty segments
        eps_tile = const_pool.tile([P, 1], f32)
        nc.vector.memset(eps_tile[:], 1e-30)

        # W^T in SBUF as bf16: column block cj holds (c_in_chunk, d) for chunk cj
        wt_tile = const_pool.tile([P, NCH * D_OUT], bf16)
        nc.sync.dma_start(
            out=wt_tile[:].rearrange("c (cj d) -> c cj d", d=D_OUT),
            in_=wfc_r,
        )

        for t in [t for _ in range(reps) for t in range(T)]:
            feats = feats_pool.tile([P, NCH * HW], bf16)
            for sp_i in range(dma_split):
                lo = sp_i * (NCH // dma_split)
                hi = (sp_i + 1) * (NCH // dma_split)
                dma_eng = nc.scalar if (sp_i % 2 == 1) else nc.sync
                dma_eng.dma_start(
                    out=feats[:, lo * HW : hi * HW].rearrange(
                        "c (cj p) -> c cj p", p=HW
                    ),
                    in_=maps_r[:, lo:hi, t, :],
                )

            if dma_only:
                # measure the pure input-DMA floor: touch feats with one tiny
                # op per frame so the loads stay live, skip all compute
                probe = small_pool.tile([P, 1], f32)
                nc.vector.reduce_sum(
                    out=probe[:], in_=feats[:, :4], axis=mybir.AxisListType.X
                )
                continue

            # mask: load (chunk, pixel) i32 -> f32, PE-transpose to (pixel, chunk)
            mask_raw = mask_pool.tile([NPIX, P], f32)
            nc.gpsimd.dma_start(out=mask_raw[:], in_=mask_r[t])
            mask_ps = mtp_pool.tile([P, NPIX], f32)
            nc.tensor.transpose(
                out=mask_ps[:], in_=mask_raw[:], identity=identity[:NPIX, :NPIX]
            )
            mask_f = maskf_pool.tile([P, NPIX], f32)
            nc.scalar.copy(out=mask_f[:], in_=mask_ps[:])

            oall = oall_pool.tile([P, NPIX * N_SP], bf16)
            nc.vector.tensor_tensor(
                out=oall[:].rearrange("p (k s) -> p k s", s=N_SP),
                in0=mask_f[:].to_broadcast([P, NPIX, N_SP]),
                in1=iota_tile[:].rearrange("p (k s) -> p k s", s=N_SP),
                op=mybir.AluOpType.is_equal,
            )

            seg = seg_pool.tile([N_SP, D_OUT], f32)
            for g in range(NPIX // 4):
                pp = pp_pool.tile([P, 4 * D_OUT], f32)
                for i in range(4):
                    ch = g * 4 + i
                    for cj in range(NCH):
                        nc.tensor.matmul(
                            out=pp[:, i * D_OUT : (i + 1) * D_OUT],
                            lhsT=feats[:, cj * HW + ch * P : cj * HW + (ch + 1) * P],
                            rhs=wt_tile[:, cj * D_OUT : (cj + 1) * D_OUT],
                            start=(cj == 0),
                            stop=(cj == NCH - 1),
                        )
                proj = proj_pool.tile([P, 4 * D_OUT], bf16)
                nc.scalar.copy(out=proj[:], in_=pp[:])
                for i in range(4):
                    ch = g * 4 + i
                    nc.tensor.matmul(
                        out=seg[:],
                        lhsT=oall[:, ch * N_SP : (ch + 1) * N_SP],
                        rhs=proj[:, i * D_OUT : (i + 1) * D_OUT],
                        start=(ch == 0),
                        stop=(ch == NPIX - 1),
                    )

            sq = small_pool.tile([N_SP, D_OUT], f32)
            ss = small_pool.tile([N_SP, 1], f32)
            nc.scalar.activation(
                out=sq[:],
                in_=seg[:],
                func=mybir.ActivationFunctionType.Square,
                accum_out=ss[:],
            )
            nrm = small_pool.tile([N_SP, 1], f32)
            nc.scalar.activation(
                out=nrm[:],
                in_=ss[:],
                func=mybir.ActivationFunctionType.Sqrt,
                bias=eps_tile[:N_SP],
            )
            inv = small_pool.tile([N_SP, 1], f32)
            nc.vector.reciprocal(out=inv[:], in_=nrm[:])
            outsb = outsb_pool.tile([N_SP, D_OUT], f32)
            nc.vector.tensor_scalar_mul(out=outsb[:], in0=seg[:], scalar1=inv[:])
            nc.sync.dma_start(out=out_t.ap()[t], in_=outsb[:])

    nc.compile()
    return nc


def _build_program_pm(reps=1, feats_bufs=3, dma_split=2, dma_only=False):
    """Pixel-major variant: host supplies maps as (T, HW, C) bf16.

    Per frame: one-hot segment-sum over raw features first
    (seg_c[s, c] = sum_p O[p, s] * feats[p, c], 32 matmuls of N=512),
    then project the tiny (100, 512) result through W^T (4 matmuls of
    N=128 after 4 PE transposes). ~210M MACs/frame vs 320M for the
    projection-first ordering, and no big PSUM->SBUF copy traffic.
    """
    from contextlib import ExitStack

    import concourse.tile as tile
    from concourse import bacc, mybir
    from concourse.masks import make_identity

    f32 = mybir.dt.float32
    bf16 = mybir.dt.bfloat16
    i32 = mybir.dt.int32

    nc = bacc.Bacc(
        "TRN2",
        target_bir_lowering=False,
        debug=False,
        num_devices=N_CORES,
    )

    maps_t = nc.dram_tensor("maps_pm", [T, HW, C], bf16, kind="ExternalInput")
    mask_t = nc.dram_tensor("sp_mask", [T, H, W], i32, kind="ExternalInput")
    wfc_t = nc.dram_tensor("W_fcT", [C, D_OUT], bf16, kind="ExternalInput")
    out_t = nc.dram_tensor("out", [T, N_SP, D_OUT], f32, kind="ExternalOutput")

    # (pixel_in_chunk, chunk, t, channel)
    maps_r = maps_t.ap().rearrange("t (k p) c -> p k t c", p=P)
    mask_r = mask_t.ap().rearrange("t h w -> t (h w)").rearrange(
        "t (k p) -> t k p", p=P
    )
    wfc_r = wfc_t.ap().rearrange("(cj c) d -> c cj d", c=P)

    with tile.TileContext(nc) as tc, ExitStack() as ctx:
        const_pool = ctx.enter_context(tc.tile_pool(name="const", bufs=1))
        feats_pool = ctx.enter_context(tc.tile_pool(name="feats", bufs=feats_bufs))
        mask_pool = ctx.enter_context(tc.tile_pool(name="mask", bufs=2))
        maskf_pool = ctx.enter_context(tc.tile_pool(name="maskf", bufs=2))
        oall_pool = ctx.enter_context(tc.tile_pool(name="oall", bufs=2))
        segsb_pool = ctx.enter_context(tc.tile_pool(name="segsb", bufs=2))
        ctsb_pool = ctx.enter_context(tc.tile_pool(name="ctsb", bufs=2))
        outsb_pool = ctx.enter_context(tc.tile_pool(name="outsb", bufs=2))
        small_pool = ctx.enter_context(tc.tile_pool(name="small", bufs=4))
        segc_pool = ctx.enter_context(tc.tile_pool(name="segc", bufs=2, space="PSUM"))
        ct_pool = ctx.enter_context(tc.tile_pool(name="ct", bufs=2, space="PSUM"))
        seg_pool = ctx.enter_context(tc.tile_pool(name="seg", bufs=2, space="PSUM"))
        mtp_pool = ctx.enter_context(tc.tile_pool(name="mtp", bufs=2, space="PSUM"))

        # iota constant (bf16: values 0..99 are exact)
        iota_tile = const_pool.tile([P, NPIX * N_SP], bf16)
        nc.gpsimd.iota(
            iota_tile[:].rearrange("p (k s) -> p k s", s=N_SP),
            pattern=[[0, NPIX], [1, N_SP]],
            base=0,
            channel_multiplier=0,
            allow_small_or_imprecise_dtypes=True,
        )

        identity = const_pool.tile([P, P], bf16)
        make_identity(nc, identity[:])

        eps_tile = const_pool.tile([P, 1], f32)
        nc.vector.memset(eps_tile[:], 1e-30)

        wt_tile = const_pool.tile([P, NCH * D_OUT], bf16)
        nc.sync.dma_start(
            out=wt_tile[:].rearrange("c (cj d) -> c cj d", d=D_OUT),
            in_=wfc_r,
        )

        for t in [t for _ in range(reps) for t in range(T)]:
            feats = feats_pool.tile([P, NPIX * C], bf16)
            for sp_i in range(dma_split):
                lo = sp_i * (NPIX // dma_split)
                hi = (sp_i + 1) * (NPIX // dma_split)
                dma_eng = nc.scalar if (sp_i % 2 == 1) else nc.sync
                dma_eng.dma_start(
                    out=feats[:, lo * C : hi * C].rearrange(
                        "p (k c) -> p k c", c=C
                    ),
                    in_=maps_r[:, lo:hi, t, :],
                )

            if dma_only:
                probe = small_pool.tile([P, 1], f32)
                nc.vector.reduce_sum(
                    out=probe[:], in_=feats[:, :4], axis=mybir.AxisListType.X
                )
                continue

            # mask: (chunk, pixel) i32 -> bf16, PE-transpose to (pixel, chunk)
            mask_raw = mask_pool.tile([NPIX, P], bf16)
            nc.gpsimd.dma_start(out=mask_raw[:], in_=mask_r[t])
            mask_ps = mtp_pool.tile([P, NPIX], bf16)
            nc.tensor.transpose(
                out=mask_ps[:], in_=mask_raw[:], identity=identity[:NPIX, :NPIX]
            )
            mask_f = maskf_pool.tile([P, NPIX], bf16)
            nc.scalar.copy(out=mask_f[:], in_=mask_ps[:])

            oall = oall_pool.tile([P, NPIX * N_SP], bf16)
            nc.vector.tensor_tensor(
                out=oall[:].rearrange("p (k s) -> p k s", s=N_SP),
                in0=mask_f[:].to_broadcast([P, NPIX, N_SP]),
                in1=iota_tile[:].rearrange("p (k s) -> p k s", s=N_SP),
                op=mybir.AluOpType.is_equal,
            )

            # stage A: per-segment channel sums, seg_c (100 s, 512 c)
            segc = segc_pool.tile([N_SP, C], f32)
            for k in range(NPIX):
                nc.tensor.matmul(
                    out=segc[:],
                    lhsT=oall[:, k * N_SP : (k + 1) * N_SP],
                    rhs=feats[:, k * C : (k + 1) * C],
                    start=(k == 0),
                    stop=(k == NPIX - 1),
                )
            segc_sb = segsb_pool.tile([N_SP, C], bf16)
            nc.scalar.copy(out=segc_sb[:], in_=segc[:])

            # transpose seg_c -> (c, s) per channel chunk, then project
            ct_sb = ctsb_pool.tile([P, NCH * N_SP], bf16)
            for cj in range(NCH):
                ctp = ct_pool.tile([P, N_SP], bf16)
                nc.tensor.transpose(
                    out=ctp[:],
                    in_=segc_sb[:, cj * P : (cj + 1) * P],
                    identity=identity[:N_SP, :N_SP],
                )
                nc.scalar.copy(
                    out=ct_sb[:, cj * N_SP : (cj + 1) * N_SP], in_=ctp[:]
                )

            # stage B: seg (100 s, 128 d) = seg_c @ W^T
            seg = seg_pool.tile([N_SP, D_OUT], f32)
            for cj in range(NCH):
                nc.tensor.matmul(
                    out=seg[:],
                    lhsT=ct_sb[:, cj * N_SP : (cj + 1) * N_SP],
                    rhs=wt_tile[:, cj * D_OUT : (cj + 1) * D_OUT],
                    start=(cj == 0),
                    stop=(cj == NCH - 1),
                )

            sq = small_pool.tile([N_SP, D_OUT], f32)
            ss = small_pool.tile([N_SP, 1], f32)
            nc.scalar.activation(
                out=sq[:],
                in_=seg[:],
                func=mybir.ActivationFunctionType.Square,
                accum_out=ss[:],
            )
            nrm = small_pool.tile([N_SP, 1], f32)
            nc.scalar.activation(
                out=nrm[:],
                in_=ss[:],
                func=mybir.ActivationFunctionType.Sqrt,
                bias=eps_tile[:N_SP],
            )
            inv = small_pool.tile([N_SP, 1], f32)
            nc.vector.reciprocal(out=inv[:], in_=nrm[:])
            outsb = outsb_pool.tile([N_SP, D_OUT], f32)
            nc.vector.tensor_scalar_mul(out=outsb[:], in0=seg[:], scalar1=inv[:])
            nc.sync.dma_start(out=out_t.ap()[t], in_=outsb[:])

    nc.compile()
    return nc


VARIANT = "cm"


def _get_program():
    if "nc" not in _CACHE:
        if VARIANT == "pm":
            _CACHE["nc"] = _build_program_pm()
        else:
            _CACHE["nc"] = _build_program()
    return _CACHE["nc"]


def kernel(maps, sp_mask, W_fc, max_sp_num):
    import ml_dtypes

    from concourse.bass_utils import run_bass_kernel_spmd

    bf16 = ml_dtypes.bfloat16
    maps = np.asarray(maps, dtype=np.float32).astype(bf16)
    sp_mask = np.asarray(sp_mask, dtype=np.int32)
    W_fc = np.asarray(W_fc, dtype=np.float32)
    assert int(max_sp_num) == N_SP
    assert maps.shape == (B, C, T, H, W)

    wt = np.ascontiguousarray(W_fc.T).astype(bf16)  # (C, D_OUT)
    nc = _get_program()
    if VARIANT == "pm":
        maps_pm = np.ascontiguousarray(
            maps.transpose(0, 2, 3, 4, 1).reshape(B, T, HW, C)
        )
        in_maps = [
            {"maps_pm": maps_pm[b], "sp_mask": sp_mask[b], "W_fcT": wt}
            for b in range(B)
        ]
    else:
        in_maps = [
            {"maps_bf16": maps[b], "sp_mask": sp_mask[b], "W_fcT": wt}
            for b in range(B)
        ]
    res = run_bass_kernel_spmd(nc, in_maps, core_ids=list(range(N_CORES)))
    # per-core out is (T, N_SP, D_OUT); full output is (B, D_OUT, T, N_SP)
    out = np.stack([res.results[b]["out"] for b in range(B)], axis=0)
    return np.ascontiguousarray(out.transpose(0, 3, 1, 2)).astype(np.float32)
